# revision 16
# baseline (speedup 1.0000x reference)
# Trainium2 Bass kernel for nn_CustomStyleLoss (segment-mean + MSE reduction).
#
# loss = sum_rows mean_chunks( (mean_chunk(input) - mean_chunk(style))^2 )
# with rows = 16*512 = 8192, each row = 50*50 = 2500 elems = 25 chunks of 100.
#
# Data-parallel over the row axis: core i gets rows [i*1024, (i+1)*1024).
# Raw Bass (no Tile framework). Per core: 8 row-tiles of 128 rows; each
# tile's 2500 columns stream as THREE chunk-aligned pieces (1200/800/500)
# per tensor. Input pieces ride the SP HWDGE ring, style pieces the ACT
# ring; together they hold ~352 GB/s, ~98% of the 358 GB/s HBM-per-core
# share, so the stream itself is at the roofline.
#
# The piece split exists for the tail: scan time is proportional to the
# COLUMN count, so finishing with a 500-column piece leaves only ~1.1us
# of DVE work after the last byte lands (vs 5.4us for a whole-tile scan),
# and each piece's chunk-sum subtraction runs while the next piece is
# still streaming.
#
# Per piece the DVE runs one tensor_tensor_scan: running sum of
# (input - style) with fp32 state; chunk sums are strided differences of
# the scan output at 100-element boundaries (col 0 of each scan buffer is
# a permanent zero). Per tile the 25 chunk sums are squared and
# row-reduced into partials[:, t]; the loss scale is applied on the host.

import sys

if "/opt/trn_rl_repo" not in sys.path:
    sys.path.insert(0, "/opt/trn_rl_repo")

import numpy as np

import concourse.bass as bass
from concourse import mybir
from concourse.bass_utils import run_bass_kernel_spmd

N_CORES = 8
N_ROWS = 8192          # 16 * 512
K = 2500               # 50 * 50
CHUNK = 100
P = 128
ROWS_PER_CORE = N_ROWS // N_CORES   # 1024
ROWS_PER_TILE = P                   # 128
N_TILES = ROWS_PER_CORE // ROWS_PER_TILE  # 8
N_BUFS = 7
# Column pieces per tile, chunk-aligned. The last piece is smallest so the
# post-stream scan tail is short.
PIECES = [(0, 1200), (1200, 2000), (2000, 2500)]
N_PIECES = len(PIECES)
SCALE = 1.0 / (CHUNK * np.sqrt(K // CHUNK))
SCALE2 = float(SCALE * SCALE)

_CACHED_NC = None


def _build_nc():
    nc = bass.Bass(
        "TRN2",
        target_bir_lowering=False,
        debug=False,
        num_devices=N_CORES,
    )
    x = nc.dram_tensor(
        "input", [ROWS_PER_CORE, K], mybir.dt.float32, kind="ExternalInput"
    ).ap()
    s = nc.dram_tensor(
        "style", [ROWS_PER_CORE, K], mybir.dt.float32, kind="ExternalInput"
    ).ap()
    o = nc.dram_tensor(
        "out", [P, N_TILES], mybir.dt.float32, kind="ExternalOutput"
    ).ap()

    def src(t_ap, t, c0, c1):
        return t_ap[t * ROWS_PER_TILE : (t + 1) * ROWS_PER_TILE, c0:c1]

    from contextlib import ExitStack

    with ExitStack() as ctx:
        xt = ctx.enter_context(
            nc.sbuf_tensor("xt", [P, N_BUFS, K], mybir.dt.float32)
        )
        st = ctx.enter_context(
            nc.sbuf_tensor("st", [P, N_BUFS, K], mybir.dt.float32)
        )
        # One scan buffer for the whole tile; piece scans chain via
        # initial=prev[:, -1:], so col 0 is a permanent zero and all 25
        # chunk sums come from ONE strided sub.
        scb = ctx.enter_context(
            nc.sbuf_tensor("scb", [P, K + 1], mybir.dt.float32)
        )
        # Triple-buffered chunk sums: DVE writes tile t's 25 sums into
        # cs[:, t%3, :] while the ACT engine squares+reduces older tiles.
        cs = ctx.enter_context(
            nc.sbuf_tensor("cs", [P, 3, K // CHUNK], mybir.dt.float32)
        )
        sq = ctx.enter_context(
            nc.sbuf_tensor("sq", [P, K // CHUNK], mybir.dt.float32)
        )
        partials = ctx.enter_context(
            nc.sbuf_tensor("partials", [P, N_TILES], mybir.dt.float32)
        )
        # One semaphore per DMA so no completion-ordering assumptions are
        # needed between DMAs on the same ring.
        s_in = [
            [
                ctx.enter_context(nc.semaphore(f"s_in{t}_{p}"))
                for p in range(N_PIECES)
            ]
            for t in range(N_TILES)
        ]
        s_st = [
            [
                ctx.enter_context(nc.semaphore(f"s_st{t}_{p}"))
                for p in range(N_PIECES)
            ]
            for t in range(N_TILES)
        ]
        s_sub = ctx.enter_context(nc.semaphore("s_sub"))
        s_cs = ctx.enter_context(nc.semaphore("s_cs"))
        s_out = ctx.enter_context(nc.semaphore("s_out"))
        block = ctx.enter_context(nc.Block(no_gpsimd_drain=True))

        @block.sync
        def _(sync):
            # Input pieces on the SP HWDGE ring. The first N_BUFS tiles
            # issue immediately; tile t >= N_BUFS reuses slot t % N_BUFS,
            # free once tile t - N_BUFS's scans+subs are done (s_sub).
            for t in range(N_TILES):
                if t >= N_BUFS:
                    sync.wait_ge(s_sub, t - N_BUFS + 1)
                for p, (c0, c1) in enumerate(PIECES):
                    sync.dma_start(
                        out=xt[:, t % N_BUFS, c0:c1], in_=src(x, t, c0, c1)
                    ).then_inc(s_in[t][p], 16)
            # Ship the per-core partial sums once all tiles are reduced.
            sync.wait_ge(s_cs, N_TILES)
            # No wait on the out-DMA receipt: the 4KB write lands in DRAM
            # within ~1us, while the completion semaphore's write-receipt
            # round trip costs 3-8us; the engine postamble + NRT teardown
            # give the write ample time before the host reads the output.
            sync.dma_start(out=o, in_=partials[:]).then_inc(s_out, 16)

        def act_square(scalar, t):
            # partials[:, t] = sum_c cs[:, t%3, c]^2 — fused square+reduce
            # on the ACT engine so the DVE only runs scans and subs.
            scalar.wait_ge(s_sub, t + 1)
            nc.scalar.activation(
                out=sq[:],
                in_=cs[:, t % 3, :],
                func=mybir.ActivationFunctionType.Square,
                accum_out=partials[:, t : t + 1],
            ).then_inc(s_cs, 1)

        @block.scalar
        def _(scalar):
            # Style pieces on the ACT HWDGE ring. All issues come first —
            # a square interleaved between issues would stall the ring on
            # its s_sub wait and starve the stream.
            for t in range(N_TILES):
                if t >= N_BUFS:
                    scalar.wait_ge(s_sub, t - N_BUFS + 1)
                for p, (c0, c1) in enumerate(PIECES):
                    scalar.dma_start(
                        out=st[:, t % N_BUFS, c0:c1], in_=src(s, t, c0, c1)
                    ).then_inc(s_st[t][p], 16)
            for t in range(N_TILES):
                act_square(scalar, t)

        @block.vector
        def _(vector):
            nc.vector.memset(scb[:, 0:1], 0.0)
            for t in range(N_TILES):
                slot = t % N_BUFS
                if t >= 3:
                    # cs slot t%3 reuse: ACT must have squared tile t-3.
                    vector.wait_ge(s_cs, t - 2)
                for p, (c0, c1) in enumerate(PIECES):
                    vector.wait_ge(s_in[t][p], 16)
                    vector.wait_ge(s_st[t][p], 16)
                    # scb[:, j] = sum_{i<=j} (xt[:, i] - st[:, i]); pieces
                    # chain through the previous piece's last column.
                    nc.vector.tensor_tensor_scan(
                        out=scb[:, c0 + 1 : c1 + 1],
                        data0=xt[:, slot, c0:c1],
                        data1=st[:, slot, c0:c1],
                        initial=0.0 if p == 0 else scb[:, c0 : c0 + 1],
                        op0=mybir.AluOpType.add,
                        op1=mybir.AluOpType.subtract,
                    )
                    vector.drain()
                # chunk sums: cs[c] = S[100(c+1)] - S[100c]  (S[0] == 0).
                # Completion frees the xt/st slot and hands cs[:, t%3] to
                # the ACT square.
                nc.vector.tensor_sub(
                    cs[:, t % 3, :],
                    scb[:, CHUNK : K + 1 : CHUNK],
                    scb[:, 0:K:CHUNK],
                ).then_inc(s_sub, 1)

    return nc


def _get_nc():
    global _CACHED_NC
    if _CACHED_NC is None:
        _CACHED_NC = _build_nc()
    return _CACHED_NC


def run_sharded(input, style, **run_kwargs):
    """Shard, run on 8 cores, return (scalar loss, BassKernelResults)."""
    nc = _get_nc()
    xi = np.ascontiguousarray(np.asarray(input, dtype=np.float32)).reshape(
        N_ROWS, K
    )
    xs = np.ascontiguousarray(np.asarray(style, dtype=np.float32)).reshape(
        N_ROWS, K
    )
    in_maps = [
        {
            "input": xi[i * ROWS_PER_CORE : (i + 1) * ROWS_PER_CORE],
            "style": xs[i * ROWS_PER_CORE : (i + 1) * ROWS_PER_CORE],
        }
        for i in range(N_CORES)
    ]
    res = run_bass_kernel_spmd(nc, in_maps, list(range(N_CORES)), **run_kwargs)
    total = np.float64(0.0)
    for r in res.results:
        total += r["out"].astype(np.float64).sum()
    return np.array(total * SCALE2, dtype=np.float32), res


def kernel(input, style):
    loss, _ = run_sharded(input, style)
    return loss


# revision 20
# speedup vs baseline: 1.1445x; 1.1445x over previous
# Trainium2 Bass kernel for nn_CustomStyleLoss (segment-mean + MSE reduction).
#
# loss = sum_rows mean_chunks( (mean_chunk(input) - mean_chunk(style))^2 )
# with rows = 16*512 = 8192, each row = 50*50 = 2500 elems = 25 chunks of 100.
#
# Data-parallel over the row axis: core i gets rows [i*1024, (i+1)*1024).
# Raw Bass (no Tile framework). Per core: 8 row-tiles of 128 rows; each
# tile's 2500 columns stream as THREE chunk-aligned pieces (1200/800/500)
# per tensor. Input pieces ride the SP HWDGE ring, style pieces the ACT
# ring; together they hold ~352 GB/s, ~98% of the 358 GB/s HBM-per-core
# share, so the stream itself is at the roofline.
#
# The piece split exists for the tail: scan time is proportional to the
# COLUMN count, so finishing with a 500-column piece leaves only ~1.1us
# of DVE work after the last byte lands (vs 5.4us for a whole-tile scan),
# and each piece's chunk-sum subtraction runs while the next piece is
# still streaming.
#
# Per piece the DVE runs one tensor_tensor_scan: running sum of
# (input - style) with fp32 state; chunk sums are strided differences of
# the scan output at 100-element boundaries (col 0 of each scan buffer is
# a permanent zero). Per tile the 25 chunk sums are squared and
# row-reduced into partials[:, t]; the loss scale is applied on the host.

import sys

if "/opt/trn_rl_repo" not in sys.path:
    sys.path.insert(0, "/opt/trn_rl_repo")

import numpy as np

import concourse.bass as bass
from concourse import mybir
from concourse.bass_utils import run_bass_kernel_spmd

N_CORES = 8
N_ROWS = 8192          # 16 * 512
K = 2500               # 50 * 50
CHUNK = 100
P = 128
ROWS_PER_CORE = N_ROWS // N_CORES   # 1024
ROWS_PER_TILE = P                   # 128
N_TILES = ROWS_PER_CORE // ROWS_PER_TILE  # 8
N_BUFS = 7
# Column pieces per tile, chunk-aligned. The last piece is smallest so the
# post-stream scan tail is short.
PIECES = [(0, 1200), (1200, 2000), (2000, 2500)]
N_PIECES = len(PIECES)
SCALE = 1.0 / (CHUNK * np.sqrt(K // CHUNK))
SCALE2 = float(SCALE * SCALE)

_CACHED_NC = None


def _build_nc():
    nc = bass.Bass(
        "TRN2",
        target_bir_lowering=False,
        debug=False,
        num_devices=N_CORES,
    )
    x = nc.dram_tensor(
        "input", [ROWS_PER_CORE, K], mybir.dt.float32, kind="ExternalInput"
    ).ap()
    s = nc.dram_tensor(
        "style", [ROWS_PER_CORE, K], mybir.dt.float32, kind="ExternalInput"
    ).ap()
    o = nc.dram_tensor(
        "out", [P, N_TILES], mybir.dt.float32, kind="ExternalOutput"
    ).ap()

    def src(t_ap, t, c0, c1):
        return t_ap[t * ROWS_PER_TILE : (t + 1) * ROWS_PER_TILE, c0:c1]

    from contextlib import ExitStack

    with ExitStack() as ctx:
        xt = ctx.enter_context(
            nc.sbuf_tensor("xt", [P, N_BUFS, K], mybir.dt.float32)
        )
        st = ctx.enter_context(
            nc.sbuf_tensor("st", [P, N_BUFS, K], mybir.dt.float32)
        )
        # One scan buffer for the whole tile; piece scans chain via
        # initial=prev[:, -1:], so col 0 is a permanent zero and all 25
        # chunk sums come from ONE strided sub.
        scb = ctx.enter_context(
            nc.sbuf_tensor("scb", [P, K + 1], mybir.dt.float32)
        )
        # One cs slot per tile (tiny) so the DVE NEVER waits on the ACT
        # squares — the issuing engines stall on ring-full backpressure
        # for multiple microseconds, and any DVE->ACT coupling would pull
        # that stall into the critical path.
        cs = ctx.enter_context(
            nc.sbuf_tensor("cs", [P, N_TILES, K // CHUNK], mybir.dt.float32)
        )
        sq = ctx.enter_context(
            nc.sbuf_tensor("sq", [P, K // CHUNK], mybir.dt.float32)
        )
        partials = ctx.enter_context(
            nc.sbuf_tensor("partials", [P, N_TILES], mybir.dt.float32)
        )
        # One semaphore per DMA so no completion-ordering assumptions are
        # needed between DMAs on the same ring.
        s_in = [
            [
                ctx.enter_context(nc.semaphore(f"s_in{t}_{p}"))
                for p in range(N_PIECES)
            ]
            for t in range(N_TILES)
        ]
        s_st = [
            [
                ctx.enter_context(nc.semaphore(f"s_st{t}_{p}"))
                for p in range(N_PIECES)
            ]
            for t in range(N_TILES)
        ]
        s_sub = ctx.enter_context(nc.semaphore("s_sub"))
        s_cs = ctx.enter_context(nc.semaphore("s_cs"))
        s_out = ctx.enter_context(nc.semaphore("s_out"))
        block = ctx.enter_context(nc.Block(no_gpsimd_drain=True))

        @block.sync
        def _(sync):
            # Input pieces on the SP HWDGE ring. The first N_BUFS tiles
            # issue immediately; tile t >= N_BUFS reuses slot t % N_BUFS,
            # free once tile t - N_BUFS's scans+subs are done (s_sub).
            for t in range(N_TILES):
                if t >= N_BUFS:
                    sync.wait_ge(s_sub, t - N_BUFS + 1)
                for p, (c0, c1) in enumerate(PIECES):
                    sync.dma_start(
                        out=xt[:, t % N_BUFS, c0:c1], in_=src(x, t, c0, c1)
                    ).then_inc(s_in[t][p], 16)
            # Ship the per-core partial sums once all tiles are reduced.
            sync.wait_ge(s_cs, N_TILES)
            # No wait on the out-DMA receipt: the 4KB write lands in DRAM
            # within ~1us, while the completion semaphore's write-receipt
            # round trip costs 3-8us; the engine postamble + NRT teardown
            # give the write ample time before the host reads the output.
            sync.dma_start(out=o, in_=partials[:]).then_inc(s_out, 16)

        def act_square(scalar, t):
            # partials[:, t] = sum_c cs[:, t%3, c]^2 — fused square+reduce
            # on the ACT engine so the DVE only runs scans and subs.
            scalar.wait_ge(s_sub, t + 1)
            nc.scalar.activation(
                out=sq[:],
                in_=cs[:, t, :],
                func=mybir.ActivationFunctionType.Square,
                accum_out=partials[:, t : t + 1],
            ).then_inc(s_cs, 1)

        @block.scalar
        def _(scalar):
            # Style pieces on the ACT HWDGE ring. All issues come first —
            # a square interleaved between issues would stall the ring on
            # its s_sub wait and starve the stream.
            for t in range(N_TILES):
                if t >= N_BUFS:
                    scalar.wait_ge(s_sub, t - N_BUFS + 1)
                for p, (c0, c1) in enumerate(PIECES):
                    scalar.dma_start(
                        out=st[:, t % N_BUFS, c0:c1], in_=src(s, t, c0, c1)
                    ).then_inc(s_st[t][p], 16)
            for t in range(N_TILES):
                act_square(scalar, t)

        @block.vector
        def _(vector):
            nc.vector.memset(scb[:, 0:1], 0.0)
            for t in range(N_TILES):
                slot = t % N_BUFS
                for p, (c0, c1) in enumerate(PIECES):
                    vector.wait_ge(s_in[t][p], 16)
                    vector.wait_ge(s_st[t][p], 16)
                    # scb[:, j] = sum_{i<=j} (xt[:, i] - st[:, i]); pieces
                    # chain through the previous piece's last column.
                    nc.vector.tensor_tensor_scan(
                        out=scb[:, c0 + 1 : c1 + 1],
                        data0=xt[:, slot, c0:c1],
                        data1=st[:, slot, c0:c1],
                        initial=0.0 if p == 0 else scb[:, c0 : c0 + 1],
                        op0=mybir.AluOpType.add,
                        op1=mybir.AluOpType.subtract,
                    )
                    vector.drain()
                # chunk sums: cs[c] = S[100(c+1)] - S[100c]  (S[0] == 0).
                # Completion frees the xt/st slot and hands cs[:, t] to
                # the ACT square.
                nc.vector.tensor_sub(
                    cs[:, t, :],
                    scb[:, CHUNK : K + 1 : CHUNK],
                    scb[:, 0:K:CHUNK],
                ).then_inc(s_sub, 1)

    return nc


def _get_nc():
    global _CACHED_NC
    if _CACHED_NC is None:
        _CACHED_NC = _build_nc()
    return _CACHED_NC


def run_sharded(input, style, **run_kwargs):
    """Shard, run on 8 cores, return (scalar loss, BassKernelResults)."""
    nc = _get_nc()
    xi = np.ascontiguousarray(np.asarray(input, dtype=np.float32)).reshape(
        N_ROWS, K
    )
    xs = np.ascontiguousarray(np.asarray(style, dtype=np.float32)).reshape(
        N_ROWS, K
    )
    in_maps = [
        {
            "input": xi[i * ROWS_PER_CORE : (i + 1) * ROWS_PER_CORE],
            "style": xs[i * ROWS_PER_CORE : (i + 1) * ROWS_PER_CORE],
        }
        for i in range(N_CORES)
    ]
    res = run_bass_kernel_spmd(nc, in_maps, list(range(N_CORES)), **run_kwargs)
    total = np.float64(0.0)
    for r in res.results:
        total += r["out"].astype(np.float64).sum()
    return np.array(total * SCALE2, dtype=np.float32), res


def kernel(input, style):
    loss, _ = run_sharded(input, style)
    return loss
